# revision 1
# baseline (speedup 1.0000x reference)
"""MambaVisionMixerBlock TRN2 Bass kernel.

Sharding: 8 cores = 2 batches x 4 sequence-quarters. Each core owns 1024
tokens of one batch and computes the full block for them, using a 127-token
left halo so the selective scan's incoming state is reproduced to below
fp32 noise (decay exp(delta*A) <= ~0.88 per step for this data;
0.88^127 ~ 1e-8).

Per-core layout: channel-major [d on partitions, tokens on free] so the
depthwise conv (3 shifted diagonal matmuls into PSUM), the scan
(DVE tensor_tensor_scan), and all per-channel scales run natively.
LayerNorm stats run token-major before a PE transpose; gamma/beta are
applied on the PSUM->SBUF eviction of the transpose (beta enters via a
K=1 rank-1 matmul with the pad-token mask so padded tokens stay exactly 0).
Matmuls run in float32r (full-rate fp32 PE path, ~1.5e-4 max rel err).

Token window per core: ext cols [o-127, o+1025) (1152 tokens), owned cols
[127, 1151). Conv-shifted processing blocks cover ext cols [-1, 1151) in
3 blocks of 384.
"""

import json as _json

import numpy as np

import concourse.bass as bass
import concourse.mybir as mybir
import concourse.tile as tile
from concourse.masks import make_identity
from concourse.vector_clock import ScopedClock, VectorClock

F32 = mybir.dt.float32
F32R = mybir.dt.float32r
AF = mybir.ActivationFunctionType
OP = mybir.AluOpType

B, L, D = 2, 4096, 1024
DS, RK, KK = 16, 64, 3
EPS = 1e-5

DEBUG_DUMP = False    # when True, dump block-0 intermediates per core

T_EXT = 1152          # tokens per core incl halo
OWN = 1024            # owned tokens per core
OWN_OFF = 127         # owned cols = [127, 1151)
NB = 3                # processing blocks
TB = 384              # block width (matmul N)
KT = D // 128         # 8 k-tiles

# ---------------------------------------------------------------------------
# Compiler workarounds: this container's walrus supports ONE sync-wait per
# instruction ("Too many sync wait commands" otherwise); Tile attaches
# several. Hoist extras onto single-wait NoOps just before the instruction
# (same engine, in-order dispatch => equivalent).
# ---------------------------------------------------------------------------

_orig_to_json_bytes = bass.Bass.to_json_bytes


def _split_waits_json(raw: bytes) -> bytes:
    d = _json.loads(raw)
    changed = False
    for fn in d.get("functions", []):
        for bb in fn.get("blocks", []):
            out = []
            for inst in bb.get("instructions", []):
                si = inst.get("sync_info")
                waits = (si or {}).get("on_wait") or []
                if len(waits) > 1:
                    for i, w in enumerate(waits[:-1]):
                        out.append({
                            "debug": inst.get("debug", 0),
                            "engine": inst["engine"],
                            "ins": [],
                            "name": f"{inst['name']}-w{i}",
                            "opcode": "NoOp",
                            "outs": [],
                            "sync_info": {"on_update": [], "on_wait": [w]},
                        })
                    si["on_wait"] = [waits[-1]]
                    changed = True
                out.append(inst)
            bb["instructions"] = out
    if not changed:
        return raw
    return _json.dumps(d).encode()


def _patched_to_json_bytes(self, *a, **k):
    return _split_waits_json(_orig_to_json_bytes(self, *a, **k))


def _patched_drain_and_barrier(self, tick_clock, wait_clock):
    nc = self.nc
    gc = tick_clock.global_clock
    n_proc = len(gc)
    for proc in range(n_proc):
        tk = gc[proc]
        if tk > 0:
            vc = VectorClock([tk if i == proc else 0 for i in range(n_proc)])
            n = nc.sync.nop(nofuse=True)
            wait_clock.add_sem_waits(n.ins, ScopedClock({None: vc}))
    nc.sync.drain()
    nc.all_engine_barrier()
    assert self.sems is not None
    popped = nc._tile_sem_poison_stack.pop()
    assert popped is self._sem_poison
    nc.clear_and_free_semaphores(list(self.sems.allocated().values()))
    nc.all_engine_barrier()


def _apply_patches():
    bass.Bass.to_json_bytes = _patched_to_json_bytes
    tile.TileContext._drain_and_barrier = _patched_drain_and_barrier


# ---------------------------------------------------------------------------
# Program builder
# ---------------------------------------------------------------------------

def build_program():
    nc = bass.Bass("TRN2", target_bir_lowering=False, debug=False, num_devices=1)

    aps = {}

    def di(name, shape, dtype):
        aps[name] = nc.dram_tensor(name, shape, dtype, kind="ExternalInput").ap()

    di("x_sl", [T_EXT, D], F32)
    di("x_res", [OWN, D], F32R)
    di("w_in", [D, 2 * D], F32R)
    di("w_ig", [D, 128], F32R)        # [W_xp | W_Bg pad | W_Cg pad]
    di("w_outer", [128, D], F32R)     # rows [W_dt(64); W_Bp(16) pad; W_Cp(16) pad]
    di("w_out", [D, D], F32R)
    di("vgamma", [128, KT], F32)
    di("vbeta", [1, D], F32)
    di("vconvb", [128, KT], F32)
    di("vbdt", [128, KT], F32)
    di("vA", [128, KT], F32)
    di("vD", [128, KT], F32)
    di("convw", [128, KT, KK], F32)
    di("mask_col", [T_EXT, 1], F32)
    di("mask_row", [1, T_EXT], F32)
    di("mask_edge", [1, 130], F32)

    aps["out"] = nc.dram_tensor("out", [OWN, D], F32, kind="ExternalOutput").ap()
    if DEBUG_DUMP:
        for nm in ["dbg_xn", "dbg_xb", "dbg_xact", "dbg_delta", "dbg_dA",
                   "dbg_dBx", "dbg_hs", "dbg_oss", "dbg_gated", "dbg_sz"]:
            aps[nm] = nc.dram_tensor(nm, [D, TB], F32,
                                     kind="ExternalOutput").ap()

    with tile.TileContext(nc) as tc:
        _build_body(nc, tc, aps)
    return nc


def _build_body(nc, tc, t):
    from contextlib import ExitStack
    es = ExitStack()
    const = es.enter_context(tc.tile_pool(name="const", bufs=1))
    sb = es.enter_context(tc.tile_pool(name="sb", bufs=2))
    # psum pools: 2+2+1+3 = 8 banks
    psA = es.enter_context(tc.tile_pool(name="psA", bufs=1, space="PSUM"))
    psB = es.enter_context(tc.tile_pool(name="psB", bufs=2, space="PSUM"))
    psC = es.enter_context(tc.tile_pool(name="psC", bufs=1, space="PSUM"))
    ps3 = es.enter_context(tc.tile_pool(name="ps3", bufs=4, space="PSUM"))

    # ---- constants -------------------------------------------------------
    ident_f = const.tile([128, 128], F32, tag="ident_f")
    make_identity(nc, ident_f[:])
    ident = const.tile([128, 128], F32R, tag="ident")
    nc.scalar.copy(ident[:], ident_f[:])

    def ldconst(name, shape, dtype, tag):
        tl = const.tile(shape, dtype, tag=tag, name=tag)
        nc.sync.dma_start(tl[:], t[name][:])
        return tl

    gamma_t = ldconst("vgamma", [128, KT], F32, "vg")
    convb_t = ldconst("vconvb", [128, KT], F32, "vcb")
    bdt_t = ldconst("vbdt", [128, KT], F32, "vbdt")
    A_t = ldconst("vA", [128, KT], F32, "vA")
    D_t = ldconst("vD", [128, KT], F32, "vD")
    beta_row = ldconst("vbeta", [1, D], F32, "vbe")
    m_row = ldconst("mask_row", [1, T_EXT], F32, "mrow")
    m_edge_row = ldconst("mask_edge", [1, 130], F32, "medg")
    convw_t = ldconst("convw", [128, KT, KK], F32, "cw")
    ones1 = const.tile([1, 128], F32, tag="ones1")
    nc.gpsimd.memset(ones1[:], 1.0)
    eps_t = const.tile([128, 1], F32, tag="eps")
    nc.gpsimd.memset(eps_t[:], EPS)

    # conv diagonal weights
    diag = []
    for kk in range(KK):
        row = []
        for d in range(KT):
            dg = const.tile([128, 128], F32R, tag=f"diag{kk}_{d}",
                            name=f"diag{kk}_{d}")
            nc.vector.tensor_scalar(dg[:], ident_f[:],
                                    convw_t[:, d, kk:kk + 1], None, OP.mult)
            row.append(dg)
        diag.append(row)

    # edge-mask broadcast [128, 129] (col i guards ext col i-1)
    ps_me = psC.tile([128, 130], F32, tag="pc")
    nc.tensor.matmul(ps_me[:], ones1[:], m_edge_row[:], start=True, stop=True)
    m_edge = const.tile([128, 130], F32, tag="medge")
    nc.scalar.copy(m_edge[:], ps_me[:])

    # weights resident (loaded once; saves 24 MB of per-block re-streaming)
    wig_t = const.tile([128, KT, 128], F32R, tag="wig")
    nc.sync.dma_start(wig_t[:], t["w_ig"].rearrange("(kt p) j -> p kt j", p=128))
    wouter_t = const.tile([128, D], F32R, tag="wouter")
    nc.sync.dma_start(wouter_t[:], t["w_outer"][:])
    woc_t = const.tile([32, D], F32R, tag="woc")
    nc.sync.dma_start(woc_t[:], t["w_outer"][96:128, :])

    # persistent cross-block carries
    hcarry = const.tile([128, KT], F32, tag="hcarry")
    xbc = const.tile([128, KT, 2], F32R, tag="xbc")
    szc = const.tile([128, KT], F32, tag="szc")
    nc.gpsimd.memset(szc[:], 0.0)

    # ---- main pipeline ---------------------------------------------------
    for tb in range(NB):
        T0 = tb * TB  # in_proj block = ext cols [T0, T0+TB)

        # -- A: LayerNorm (token-major) -----------------------------------
        xhat = []
        for tt in range(3):
            ts0 = T0 + tt * 128
            x_tm = sb.tile([128, D], F32, tag="x_tm", name="x_tm")
            nc.sync.dma_start(x_tm[:], t["x_sl"][ts0:ts0 + 128, :])
            xh = sb.tile([128, D], F32R, tag="xhat", bufs=3, name="xhat")
            scratch = sb.tile([128, D], F32, tag="scratch", bufs=1, name="scr")
            sx = sb.tile([128, 1], F32, tag="sx", name="sx")
            nc.scalar.activation(scratch[:], x_tm[:], AF.Identity,
                                 accum_out=sx[:])
            sq = sb.tile([128, 1], F32, tag="sq", name="sq")
            nc.scalar.activation(scratch[:], x_tm[:], AF.Square,
                                 accum_out=sq[:])
            negmu = sb.tile([128, 1], F32, tag="negmu", name="negmu")
            nc.vector.tensor_scalar(negmu[:], sx[:], -1.0 / D, None, OP.mult)
            mu = sb.tile([128, 1], F32, tag="mu", name="mu")
            nc.vector.tensor_scalar(mu[:], sx[:], 1.0 / D, None, OP.mult)
            msq = sb.tile([128, 1], F32, tag="msq", name="msq")
            nc.vector.tensor_scalar(msq[:], sq[:], 1.0 / D, None, OP.mult)
            var = sb.tile([128, 1], F32, tag="var", name="var")
            nc.vector.scalar_tensor_tensor(var[:], mu[:], negmu[:], msq[:],
                                           OP.mult, OP.add)
            lnv = sb.tile([128, 1], F32, tag="lnv", name="lnv")
            nc.scalar.activation(lnv[:], var[:], AF.Ln, bias=eps_t[:])
            sinv = sb.tile([128, 1], F32, tag="sinv", name="sinv")
            nc.scalar.activation(sinv[:], lnv[:], AF.Exp, scale=-0.5)
            m_t = sb.tile([128, 1], F32, tag="m_t", name="m_t")
            nc.sync.dma_start(m_t[:], t["mask_col"][ts0:ts0 + 128, :])
            sc_eff = sb.tile([128, 1], F32, tag="sc_eff", name="sc_eff")
            nc.vector.tensor_mul(sc_eff[:], sinv[:], m_t[:])
            bi_eff = sb.tile([128, 1], F32, tag="bi_eff", name="bi_eff")
            nc.vector.tensor_mul(bi_eff[:], negmu[:], sc_eff[:])
            nc.scalar.activation(xh[:], x_tm[:], AF.Identity,
                                 bias=bi_eff[:], scale=sc_eff[:])
            xhat.append(xh)

        # -- B: transpose to channel-major + gamma/beta -------------------
        xn = []
        for d in range(KT):
            ps_x = psA.tile([128, TB], F32R, tag="pA", name="ps_x")
            for tt in range(3):
                nc.tensor.matmul(ps_x[:, tt * 128:(tt + 1) * 128],
                                 xhat[tt][:, d * 128:(d + 1) * 128],
                                 ident[:], is_transpose=True,
                                 start=(tt == 0), stop=False,
                                 skip_group_check=True)
            nc.tensor.matmul(ps_x[:].bitcast(F32),
                             beta_row[:, d * 128:(d + 1) * 128],
                             m_row[:, T0:T0 + TB], start=False, stop=True,
                             skip_group_check=True)
            xn_d = sb.tile([128, TB], F32R, tag="xn", bufs=9, name="xn")
            nc.scalar.activation(xn_d[:], ps_x[:].bitcast(F32), AF.Identity,
                                 scale=gamma_t[:, d:d + 1])
            if DEBUG_DUMP and tb == 0:
                nc.sync.dma_start(t["dbg_xn"][d * 128:(d + 1) * 128, :],
                                  xn_d[:].bitcast(F32))
            xn.append(xn_d)

        # -- C: in_proj xb half (streamed weights, 512-col j-groups) ------
        xb = []
        xact = []
        for j in range(KT):
            if j % 4 == 0:
                win = []
                for k in range(KT):
                    w_t = sb.tile([128, 512], F32R, tag="win", bufs=9, name="win")
                    nc.sync.dma_start(
                        w_t[:], t["w_in"][k * 128:(k + 1) * 128,
                                          (j // 4) * 512:(j // 4 + 1) * 512])
                    win.append(w_t)
            ps_xb = psB.tile([128, TB], F32, tag="pB", name="ps_xb")
            for k in range(KT):
                nc.tensor.matmul(ps_xb[:],
                                 win[k][:, (j % 4) * 128:(j % 4 + 1) * 128],
                                 xn[k][:], start=(k == 0),
                                 stop=(k == KT - 1))
            xb_d = sb.tile([128, TB + 2], F32R, tag="xb", bufs=3, name="xb")
            if tb == 0:
                nc.vector.memset(xb_d[:, 0:2].bitcast(F32), 0.0)
            else:
                nc.vector.tensor_copy(xb_d[:, 0:2], xbc[:, j, :])
            nc.scalar.copy(xb_d[:, 2:TB + 2], ps_xb[:])
            nc.vector.tensor_copy(xbc[:, j, :], xb_d[:, TB:TB + 2])
            if DEBUG_DUMP and tb == 0:
                nc.sync.dma_start(t["dbg_xb"][j * 128:(j + 1) * 128, :],
                                  xb_d[:, 2:TB + 2].bitcast(F32))
            xb.append(xb_d)

            # depthwise conv for this d-tile (3 diagonal matmuls) + SiLU
            d = j
            ps_c = psC.tile([128, TB], F32, tag="pc", name="ps_c")
            for kk in range(KK):
                nc.tensor.matmul(ps_c[:], diag[kk][d][:],
                                 xb_d[:, kk:kk + TB],
                                 start=(kk == 0), stop=(kk == KK - 1))
            if tb == 0:
                nc.vector.tensor_mul(ps_c[:, 0:128], ps_c[:, 0:128],
                                     m_edge[:, 0:128])
            xa = sb.tile([128, TB], F32R, tag="xact", bufs=8, name="xact")
            nc.scalar.activation(xa[:], ps_c[:], AF.Silu,
                                 bias=convb_t[:, d:d + 1])
            if DEBUG_DUMP and tb == 0:
                nc.sync.dma_start(t["dbg_xact"][d * 128:(d + 1) * 128, :],
                                  xa[:].bitcast(F32))
            xact.append(xa)

        # -- E: inner projections -----------------------------------------
        ps_i = psA.tile([128, TB], F32, tag="pA", name="ps_i")
        for k in range(KT):
            nc.tensor.matmul(ps_i[:], wig_t[:, k, :], xact[k][:],
                             start=(k == 0), stop=(k == KT - 1))
        inner = sb.tile([128, TB], F32R, tag="inner", bufs=1, name="inner")
        nc.scalar.copy(inner[:], ps_i[:])
        inner_c = sb.tile([32, TB], F32R, tag="inner_c", bufs=1, name="inner_c")
        nc.scalar.copy(inner_c[:], ps_i[96:128, :])

        # -- F..J: per-d-tile SSM -----------------------------------------
        gated = []
        for d in range(KT):
            dsl = slice(d * 128, (d + 1) * 128)
            ps_dt = ps3.tile([128, TB], F32, tag="p3", name="ps_dt")
            nc.tensor.matmul(ps_dt[:], wouter_t[0:64, dsl], inner[0:64, :],
                             start=True, stop=True)
            ps_B = ps3.tile([128, TB], F32, tag="p3", name="ps_B")
            nc.tensor.matmul(ps_B[:], wouter_t[64:96, dsl], inner[64:96, :],
                             start=True, stop=True)
            ps_C = ps3.tile([128, TB], F32, tag="p3", name="ps_C")
            nc.tensor.matmul(ps_C[:], woc_t[:, dsl], inner_c[:],
                             start=True, stop=True)

            e_t = sb.tile([128, TB], F32, tag="w1", name="e_t")
            nc.scalar.activation(e_t[:], ps_dt[:], AF.Exp,
                                 bias=bdt_t[:, d:d + 1])
            delta = sb.tile([128, TB], F32, tag="delta", name="delta")
            nc.scalar.activation(delta[:], e_t[:], AF.Ln, bias=1.0)
            dA = sb.tile([128, TB], F32, tag="dA", name="dA")
            nc.scalar.activation(dA[:], delta[:], AF.Exp,
                                 scale=A_t[:, d:d + 1])
            tmp1 = sb.tile([128, TB], F32, tag="w2", name="tmp1")
            nc.vector.tensor_mul(tmp1[:], ps_B[:], delta[:])
            dBx = sb.tile([128, TB], F32, tag="dBx", name="dBx")
            nc.vector.tensor_mul(dBx[:], tmp1[:], xact[d][:].bitcast(F32))

            hs = sb.tile([128, TB], F32, tag="hs", name="hs")
            init = 0.0 if tb == 0 else hcarry[:, d:d + 1]
            nc.vector.tensor_tensor_scan(hs[:], dA[:], dBx[:], init,
                                         OP.mult, OP.add)
            nc.vector.tensor_copy(hcarry[:, d:d + 1], hs[:, TB - 1:TB])

            y = sb.tile([128, TB], F32, tag="w1", name="y")
            nc.vector.tensor_mul(y[:], ps_C[:], hs[:])
            oss = sb.tile([128, TB], F32R, tag="oss", bufs=9, name="oss")
            nc.vector.scalar_tensor_tensor(oss[:],
                                           xact[d][:].bitcast(F32),
                                           D_t[:, d:d + 1], y[:],
                                           OP.mult, OP.add)
            if DEBUG_DUMP and tb == 0:
                for nm, tl in [("dbg_delta", delta), ("dbg_dA", dA),
                               ("dbg_dBx", dBx), ("dbg_hs", hs)]:
                    nc.sync.dma_start(t[nm][d * 128:(d + 1) * 128, :], tl[:])
                nc.sync.dma_start(t["dbg_oss"][d * 128:(d + 1) * 128, :],
                                  oss[:].bitcast(F32))
            gated.append(oss)

        # -- K: z half of in_proj + gating (in-place on oss) --------------
        for j in range(KT):
            if j % 4 == 0:
                winz = []
                for k in range(KT):
                    w_t = sb.tile([128, 512], F32R, tag="win", bufs=9, name="winz")
                    nc.sync.dma_start(
                        w_t[:], t["w_in"][k * 128:(k + 1) * 128,
                                          D + (j // 4) * 512:D + (j // 4 + 1) * 512])
                    winz.append(w_t)
            if True:
                ps_z = psB.tile([128, TB], F32, tag="pB", name="ps_z")
                for k in range(KT):
                    nc.tensor.matmul(ps_z[:],
                                     winz[k][:, (j % 4) * 128:(j % 4 + 1) * 128],
                                     xn[k][:], start=(k == 0),
                                     stop=(k == KT - 1))
                sz = sb.tile([128, TB], F32, tag="sz", name="sz")
                nc.scalar.activation(sz[:], ps_z[:], AF.Silu)
                # oss/gated live on the conv-shifted grid (ext col T0-1+c);
                # sz lives on the in_proj grid (ext col T0+c): multiply with
                # a one-column shift, carrying sz's last column across blocks.
                g2 = gated[j]
                nc.vector.tensor_mul(g2[:, 1:TB], g2[:, 1:TB].bitcast(F32),
                                     sz[:, 0:TB - 1])
                nc.vector.tensor_mul(g2[:, 0:1], g2[:, 0:1].bitcast(F32),
                                     szc[:, j:j + 1])
                nc.vector.tensor_copy(szc[:, j:j + 1], sz[:, TB - 1:TB])
                if DEBUG_DUMP and tb == 0:
                    nc.sync.dma_start(t["dbg_sz"][j * 128:(j + 1) * 128, :],
                                      sz[:])
                if DEBUG_DUMP and tb == 0:
                    nc.sync.dma_start(t["dbg_gated"][j * 128:(j + 1) * 128, :],
                                      gated[j][:].bitcast(F32))

        # -- L: out_proj + residual for owned cols of this block ----------
        own_lo = max(OWN_OFF, T0 - 1)
        own_hi = min(OWN_OFF + OWN, T0 + TB - 1)
        for h in range(2):
            wo = []
            for k in range(KT):
                wo_t = sb.tile([128, 512], F32R, tag="wout", bufs=9, name="wo")
                nc.sync.dma_start(
                    wo_t[:], t["w_out"][k * 128:(k + 1) * 128,
                                        h * 512:(h + 1) * 512])
                wo.append(wo_t)
            s0 = own_lo
            while s0 < own_hi:
                xr = sb.tile([128, 512], F32R, tag="xr", bufs=1, name="xr")
                nc.sync.dma_start(
                    xr[:], t["x_res"][s0 - OWN_OFF:s0 - OWN_OFF + 128,
                                      h * 512:(h + 1) * 512])
                ps_o = psB.tile([128, 512], F32, tag="pB", name="ps_o")
                for k in range(KT):
                    nc.tensor.matmul(
                        ps_o[:],
                        gated[k][:, s0 - (T0 - 1):s0 - (T0 - 1) + 128],
                        wo[k][:, :], start=(k == 0), stop=False)
                nc.tensor.matmul(ps_o[:], ident[:], xr[:],
                                 start=False, stop=True)
                o_t = sb.tile([128, 512], F32, tag="o_t", bufs=1, name="o_t")
                nc.scalar.copy(o_t[:], ps_o[:])
                nc.sync.dma_start(
                    t["out"][s0 - OWN_OFF:s0 - OWN_OFF + 128,
                             h * 512:(h + 1) * 512], o_t[:])
                s0 += 128

    es.close()


# ---------------------------------------------------------------------------
# Host-side driver
# ---------------------------------------------------------------------------

_NC_CACHE = None


def _get_program():
    global _NC_CACHE
    if _NC_CACHE is None:
        _apply_patches()
        _NC_CACHE = build_program()
    return _NC_CACHE


def _prep_inputs(x, gamma, beta, W_in, conv_w, conv_b, W_xp, W_Bg, W_Cg,
                 W_dt, b_dt, W_Bp, W_Cp, A, D_skip, W_out):
    f = lambda v: np.ascontiguousarray(np.asarray(v, dtype=np.float32))
    x = f(x)
    vec = lambda v: np.ascontiguousarray(f(v).reshape(KT, 128).T)  # [128, KT]

    w_ig = np.zeros((D, 128), np.float32)
    w_ig[:, 0:64] = f(W_xp)
    w_ig[:, 64:80] = f(W_Bg)
    w_ig[:, 96:112] = f(W_Cg)
    w_outer = np.zeros((128, D), np.float32)
    w_outer[0:64, :] = f(W_dt)
    w_outer[64:80, :] = f(W_Bp)
    w_outer[96:112, :] = f(W_Cp)

    convw = f(conv_w).reshape(KK, D)  # [3, 1024]
    convw_t = np.ascontiguousarray(
        convw.T.reshape(KT, 128, KK).transpose(1, 0, 2))  # [128, KT, 3]

    shared = {
        "w_in": f(W_in),
        "w_ig": w_ig,
        "w_outer": w_outer,
        "w_out": f(W_out),
        "vgamma": vec(gamma),
        "vbeta": f(beta).reshape(1, D),
        "vconvb": vec(conv_b),
        "vbdt": vec(b_dt),
        "vA": vec(A),
        "vD": vec(D_skip),
        "convw": convw_t,
    }

    in_maps = []
    for core in range(8):
        b, q = divmod(core, 4)
        o = q * OWN
        lo = o - OWN_OFF
        xs = np.zeros((T_EXT, D), np.float32)
        mk = np.zeros((T_EXT,), np.float32)
        s_lo, s_hi = max(0, lo), min(L, lo + T_EXT)
        xs[s_lo - lo:s_hi - lo] = x[b, s_lo:s_hi]
        mk[s_lo - lo:s_hi - lo] = 1.0
        me = np.zeros((130,), np.float32)
        me[0] = 1.0 if 0 <= lo - 1 < L else 0.0
        me[1:128] = mk[0:127]
        in_maps.append({
            **shared,
            "x_sl": xs,
            "x_res": np.ascontiguousarray(x[b, o:o + OWN]),
            "mask_col": mk.reshape(T_EXT, 1).copy(),
            "mask_row": mk.reshape(1, T_EXT).copy(),
            "mask_edge": me.reshape(1, 130),
        })
    return in_maps


def kernel(**inputs):
    from concourse.bass_utils import run_bass_kernel_spmd
    nc = _get_program()
    in_maps = _prep_inputs(**inputs)
    res = run_bass_kernel_spmd(nc, in_maps, core_ids=list(range(8)))
    out = np.empty((B, L, D), np.float32)
    for core in range(8):
        b, q = divmod(core, 4)
        out[b, q * OWN:(q + 1) * OWN, :] = res.results[core]["out"]
    return out



# revision 50
# speedup vs baseline: 22849.9490x; 22849.9490x over previous
"""MambaVisionMixerBlock TRN2 Bass kernel (v2).

Sharding: 8 cores = 2 batches x 4 sequence-quarters. Each core owns 1024
tokens of one batch and computes the full block for them, using a 127-token
left halo so the selective scan's incoming state is reproduced to below
tolerance (decay exp(delta*A) per step; halo error decays ~0.88^127).

v2 layout/strategy (vs v1):
  - full 1152-token extent processed in one pass (scan init=0, no carries)
  - all weights resident in SBUF, bf16 (same PE rate, half the HBM/SBUF)
  - scalar-engine activations batched by table set (ln_exp -> silu -> ln_exp)
  - PSUM evictions distributed across DVE/gpsimd/scalar by phase load
  - LN interleaved with in_proj by 384-token chunks to keep the PE warm
  - residual applied on DVE; z-half computed for owned tokens only
"""

import json as _json

import numpy as np

import concourse.bass as bass
import concourse.mybir as mybir
import concourse.tile as tile
from concourse.masks import make_identity
from concourse.vector_clock import ScopedClock, VectorClock

F32 = mybir.dt.float32
F32R = mybir.dt.float32r
BF16 = mybir.dt.bfloat16
AF = mybir.ActivationFunctionType
OP = mybir.AluOpType

B, L, D = 2, 4096, 1024
DS, RK, KK = 16, 64, 3
EPS = 1e-5

T_EXT = 1152          # tokens per core incl halo
OWN = 1024            # owned tokens per core
OFF = 127             # owned ext cols = [127, 1151)
CH = 384              # ext-grid chunk width
KT = D // 128         # 8 d-tiles

DEBUG_DUMP = False

# ---------------------------------------------------------------------------
# Compiler workarounds: this container's walrus supports ONE sync-wait per
# instruction; Tile attaches several. Hoist extras onto single-wait NoOps.
# ---------------------------------------------------------------------------

_orig_to_json_bytes = bass.Bass.to_json_bytes


def _split_waits_json(raw: bytes) -> bytes:
    d = _json.loads(raw)
    changed = False
    for fn in d.get("functions", []):
        for bb in fn.get("blocks", []):
            out = []
            for inst in bb.get("instructions", []):
                si = inst.get("sync_info")
                waits = (si or {}).get("on_wait") or []
                if len(waits) > 1:
                    for i, w in enumerate(waits[:-1]):
                        out.append({
                            "debug": inst.get("debug", 0),
                            "engine": inst["engine"],
                            "ins": [],
                            "name": f"{inst['name']}-w{i}",
                            "opcode": "NoOp",
                            "outs": [],
                            "sync_info": {"on_update": [], "on_wait": [w]},
                        })
                    si["on_wait"] = [waits[-1]]
                    changed = True
                out.append(inst)
            bb["instructions"] = out
    if not changed:
        return raw
    return _json.dumps(d).encode()


def _patched_to_json_bytes(self, *a, **k):
    return _split_waits_json(_orig_to_json_bytes(self, *a, **k))


def _patched_drain_and_barrier(self, tick_clock, wait_clock):
    nc = self.nc
    gc = tick_clock.global_clock
    n_proc = len(gc)
    for proc in range(n_proc):
        tk = gc[proc]
        if tk > 0:
            vc = VectorClock([tk if i == proc else 0 for i in range(n_proc)])
            n = nc.sync.nop(nofuse=True)
            wait_clock.add_sem_waits(n.ins, ScopedClock({None: vc}))
    nc.sync.drain()
    nc.all_engine_barrier()
    assert self.sems is not None
    popped = nc._tile_sem_poison_stack.pop()
    assert popped is self._sem_poison
    nc.clear_and_free_semaphores(list(self.sems.allocated().values()))
    nc.all_engine_barrier()


def _apply_patches():
    bass.Bass.to_json_bytes = _patched_to_json_bytes
    tile.TileContext._drain_and_barrier = _patched_drain_and_barrier


# ---------------------------------------------------------------------------
# Program builder
# ---------------------------------------------------------------------------

def build_program():
    nc = bass.Bass("TRN2", target_bir_lowering=False, debug=False, num_devices=1)

    aps = {}

    def di(name, shape, dtype):
        aps[name] = nc.dram_tensor(name, shape, dtype, kind="ExternalInput").ap()

    di("x_sl", [T_EXT, D], F32)      # f32 x window (residual reads only)
    di("x_bf", [T_EXT, D], BF16)     # bf16 x window (LN reads; 2.3 MB not 4.7)
    di("w_x", [128, KT * D], BF16)   # in_proj xb half, p-major packed
    di("w_z", [128, KT * D], BF16)   # in_proj z half, p-major packed
    di("w_o", [128, KT * D], BF16)   # out_proj, p-major packed
    di("w_ig", [128, KT * 128], BF16)  # [W_xp | W_Bg pad | W_Cg pad], p-major
    di("w_ab", [128, D], BF16)       # rows [W_dt(64); W_Bp(16) pad to 96; 0]
    di("w_c", [32, D], BF16)         # rows [W_Cp(16); 0]
    # all small per-channel vectors packed into one DMA:
    # cols [0:8]=gamma [8:16]=beta [16:24]=convb [24:32]=bdt [32:40]=A
    # [40:48]=D [48:72]=convw(d*3+kk) [72:81]=mask_col
    di("vpack", [128, 81], F32)
    di("medge_row", [1, 130], F32)   # col c = mask of ext col c-1 (conv grid)
    di("mbl_row", [1, 128], F32)     # mask of ext cols [0,128)
    di("mbr_row", [1, 128], F32)     # mask of ext cols [1024,1152)

    aps["out"] = nc.dram_tensor("out", [OWN, D], F32, kind="ExternalOutput").ap()
    if DEBUG_DUMP:
        for nm, w in [("dbg_xn", T_EXT), ("dbg_xb", T_EXT + 2),
                      ("dbg_xact", T_EXT), ("dbg_delta", T_EXT),
                      ("dbg_dA", T_EXT), ("dbg_dBx", T_EXT),
                      ("dbg_hs", T_EXT), ("dbg_gated", OWN),
                      ("dbg_sz", OWN)]:
            aps[nm] = nc.dram_tensor(nm, [128, w], F32,
                                     kind="ExternalOutput").ap()

    with tile.TileContext(nc) as tc:
        _build_body(nc, tc, aps)
    return nc


def _build_body(nc, tc, t):
    from contextlib import ExitStack
    es = ExitStack()
    const = es.enter_context(tc.tile_pool(name="const", bufs=1))
    sb = es.enter_context(tc.tile_pool(name="sb", bufs=2))
    psT = es.enter_context(tc.tile_pool(name="psT", bufs=2, space="PSUM"))
    psMM = es.enter_context(tc.tile_pool(name="psMM", bufs=4, space="PSUM"))
    psS = es.enter_context(tc.tile_pool(name="psS", bufs=2, space="PSUM"))

    # ---- constants -------------------------------------------------------
    ident_f = const.tile([128, 128], F32, tag="ident_f")
    make_identity(nc, ident_f[:])
    ident_b = const.tile([128, 128], BF16, tag="ident_b")
    nc.vector.tensor_copy(ident_b[:], ident_f[:])

    def ldconst(name, shape, dtype, tag):
        tl = const.tile(shape, dtype, tag=tag, name=tag)
        nc.sync.dma_start(tl[:], t[name][:])
        return tl

    vpack_t = const.tile([128, 81], F32, tag="vpack", name="vpack")
    nc.scalar.dma_start(vpack_t[:], t["vpack"][:])
    gam_t = vpack_t[:, 0:8]
    bet_t = vpack_t[:, 8:16]
    cvb_t = vpack_t[:, 16:24]
    bdt_t = vpack_t[:, 24:32]
    A_t = vpack_t[:, 32:40]
    D_t = vpack_t[:, 40:48]
    convw_t = vpack_t[:, 48:72].rearrange("p (kt kk) -> p kt kk", kt=KT)
    mcol_t = vpack_t[:, 72:81]
    medge_row = ldconst("medge_row", [1, 130], F32, "merow")
    mbl_row = ldconst("mbl_row", [1, 128], F32, "mblrow")
    mbr_row = ldconst("mbr_row", [1, 128], F32, "mbrrow")
    ones1 = const.tile([1, 128], F32, tag="ones1")
    nc.gpsimd.memset(ones1[:], 1.0)
    eps_t = const.tile([128, 1], F32, tag="eps")
    nc.gpsimd.memset(eps_t[:], EPS)

    # resident weights (bf16, loaded once). Issued on the scalar (Activation)
    # HWDGE queue, in order of first use, so x/LN traffic on the sync queue
    # isn't stuck behind 6.6 MB of weights.
    wx_t = const.tile([128, KT, D], BF16, tag="wx")
    wig_t = const.tile([128, KT, 128], BF16, tag="wig")
    wz_t = const.tile([128, KT, D], BF16, tag="wz")
    wab_t = const.tile([128, D], BF16, tag="wab")
    nc.scalar.dma_start(wab_t[:], t["w_ab"][:])
    wc_t = const.tile([32, D], BF16, tag="wc")
    nc.scalar.dma_start(wc_t[:], t["w_c"][:])
    wo_t = const.tile([128, KT, D], BF16, tag="wo")

    # broadcast masks ([128, n] copies of row masks) via K=1 matmuls
    def bcast_mask(row_t, n, tag):
        ps = psMM.tile([128, 512], F32, tag="pMM", name=f"ps_{tag}")
        nc.tensor.matmul(ps[:, 0:n], ones1[:], row_t[:],
                         start=True, stop=True)
        mb = const.tile([128, n], F32, tag=tag)
        nc.vector.tensor_copy(mb[:], ps[:, 0:n])
        return mb

    # ---- persistent activations -----------------------------------------
    xn = [sb.tile([128, T_EXT], BF16, tag="xn", bufs=KT, name=f"xn{d}")
          for d in range(KT)]
    xb = [sb.tile([128, T_EXT + 2], BF16, tag="xb", bufs=KT, name=f"xb{d}")
          for d in range(KT)]
    xact = [sb.tile([128, T_EXT], BF16, tag="xact", bufs=KT, name=f"xact{d}")
            for d in range(KT)]
    sz = [sb.tile([128, OWN], BF16, tag="sz", bufs=KT, name=f"sz{d}")
          for d in range(KT)]
    gated = [sb.tile([128, OWN], BF16, tag="gated", bufs=KT, name=f"gated{d}")
             for d in range(KT)]
    for d in range(KT):
        nc.vector.memset(xb[d][:, 0:2], 0.0)

    # ---- phase A: LN + transpose + in_proj xb, by 384-token chunk -------
    def ln_chunk(g):
        if g == 1:
            nc.scalar.dma_start(
                wz_t[:], t["w_z"].rearrange("p (kt j) -> p kt j", kt=KT))
        if g == 2:
            nc.scalar.dma_start(
                wo_t[:], t["w_o"].rearrange("p (kt j) -> p kt j", kt=KT))
        xhat = []
        for tt in range(3):
            ts0 = g * CH + tt * 128
            x_tm = sb.tile([128, D], BF16, tag="x_tm", name="x_tm")
            (nc.scalar if g == 0 else nc.sync).dma_start(
                x_tm[:], t["x_bf"][ts0:ts0 + 128, :])
            if g == 0 and tt == 2:
                nc.scalar.dma_start(
                    wx_t[:], t["w_x"].rearrange("p (kt j) -> p kt j", kt=KT))
                nc.scalar.dma_start(
                    wig_t[:], t["w_ig"].rearrange("p (kt j) -> p kt j", kt=KT))
            scratch = sb.tile([128, D], BF16, tag="scr", bufs=1, name="scr")
            sx = sb.tile([128, 1], F32, tag="sx", name="sx")
            nc.vector.tensor_reduce(sx[:], x_tm[:], mybir.AxisListType.XYZW,
                                    OP.add)
            sq = sb.tile([128, 1], F32, tag="sq", name="sq")
            nc.scalar.activation(scratch[:], x_tm[:], AF.Square,
                                 accum_out=sq[:])
            negmu = sb.tile([128, 1], F32, tag="negmu", name="negmu")
            nc.vector.tensor_scalar(negmu[:], sx[:], -1.0 / D, None, OP.mult)
            mu = sb.tile([128, 1], F32, tag="mu", name="mu")
            nc.vector.tensor_scalar(mu[:], sx[:], 1.0 / D, None, OP.mult)
            msq = sb.tile([128, 1], F32, tag="msq", name="msq")
            nc.vector.tensor_scalar(msq[:], sq[:], 1.0 / D, None, OP.mult)
            var = sb.tile([128, 1], F32, tag="var", name="var")
            nc.vector.scalar_tensor_tensor(var[:], mu[:], negmu[:], msq[:],
                                           OP.mult, OP.add)
            lnv = sb.tile([128, 1], F32, tag="lnv", name="lnv")
            nc.scalar.activation(lnv[:], var[:], AF.Ln, bias=eps_t[:])
            sinv = sb.tile([128, 1], F32, tag="sinv", name="sinv")
            nc.scalar.activation(sinv[:], lnv[:], AF.Exp, scale=-0.5)
            mi = g * 3 + tt
            sc_eff = sb.tile([128, 1], F32, tag="sc_eff", name="sc_eff")
            nc.vector.tensor_mul(sc_eff[:], sinv[:], mcol_t[:, mi:mi + 1])
            bi_eff = sb.tile([128, 1], F32, tag="bi_eff", name="bi_eff")
            nc.vector.tensor_mul(bi_eff[:], negmu[:], sc_eff[:])
            xh = sb.tile([128, D], BF16, tag="xhat", bufs=3, name="xhat")
            nc.vector.tensor_scalar(xh[:], x_tm[:], sc_eff[:], bi_eff[:],
                                    OP.mult, OP.add)
            xhat.append(xh)
        return xhat

    def transpose_one(g, xhat, d):
        # channel-major transpose + gamma/beta for one d-tile of chunk g
        ps_x = psT.tile([128, CH], BF16, tag="pT", name="ps_x")
        for tt in range(3):
            nc.tensor.matmul(ps_x[:, tt * 128:(tt + 1) * 128],
                             xhat[tt][:, d * 128:(d + 1) * 128],
                             ident_b[:], is_transpose=True,
                             start=True, stop=True,
                             skip_group_check=True)
        nc.vector.tensor_scalar(xn[d][:, g * CH:(g + 1) * CH], ps_x[:],
                                gam_t[:, d:d + 1], bet_t[:, d:d + 1],
                                OP.mult, OP.add)

    def inproj_one(g, j):
        ps = psMM.tile([128, 512], F32, tag="pMM", name="ps_xb")
        for k in range(KT):
            nc.tensor.matmul(ps[:, 0:CH], wx_t[:, k, j * 128:(j + 1) * 128],
                             xn[k][:, g * CH:(g + 1) * CH],
                             start=(k == 0), stop=(k == KT - 1),
                             skip_group_check=True)
        nc.vector.tensor_copy(xb[j][:, 2 + g * CH:2 + (g + 1) * CH],
                              ps[:, 0:CH])

    # chunk 0: LN + transposes up front; chunks 1-2: the next chunk's
    # transposes are interleaved into the current chunk's in_proj j-loop in
    # <=2-tile bursts so transpose-mode (which HAM ignores) never forms a
    # >3.4us "idle" window that re-throttles the PE clock.
    xhat_g = ln_chunk(0)
    for d in range(KT):
        transpose_one(0, xhat_g, d)
    for g in range(3):
        xhat_next = ln_chunk(g + 1) if g < 2 else None
        for j in range(KT):
            inproj_one(g, j)
            if xhat_next is not None and j % 2 == 1:
                d0 = (j // 2) * 2
                transpose_one(g + 1, xhat_next, d0)
                transpose_one(g + 1, xhat_next, d0 + 1)

    # conv diagonal weights (scalar-engine Copy with per-partition scale;
    # Copy is in every ACT table set so this causes no table reload)
    diag = []
    for kk in range(KK):
        row = []
        for d in range(KT):
            dg = const.tile([128, 128], BF16, tag=f"diag{kk}_{d}",
                            name=f"diag{kk}_{d}")
            nc.scalar.activation(dg[:], ident_f[:], AF.Copy,
                                 scale=convw_t[:, d, kk:kk + 1])
            row.append(dg)
        diag.append(row)

    medge_b = bcast_mask(medge_row, 130, "medgeb")
    mbl_b = bcast_mask(mbl_row, 128, "mblb")
    mbr_b = bcast_mask(mbr_row, 128, "mbrb")

    # pad-edge zeroing of xn (general-beta safety; in-place DVE muls)
    for d in range(KT):
        nc.vector.tensor_mul(xn[d][:, 0:128], xn[d][:, 0:128], mbl_b[:])
        nc.vector.tensor_mul(xn[d][:, OWN:T_EXT], xn[d][:, OWN:T_EXT],
                             mbr_b[:])
        if DEBUG_DUMP and d == 0:
            nc.sync.dma_start(t["dbg_xn"][:], xn[d][:].cast_dma(F32))

    # ---- phase B: depthwise conv + SiLU, then inner projections ---------
    for d in range(KT):
        pcs = [psMM.tile([128, 512], F32, tag="pMM", name=f"pc{c}")
               for c in range(3)]
        for kk in range(KK):
            for c in range(3):
                nc.tensor.matmul(pcs[c][:, 0:CH], diag[kk][d][:],
                                 xb[d][:, c * CH + kk:c * CH + kk + CH],
                                 start=(kk == 0), stop=(kk == KK - 1),
                                 skip_group_check=True)
        for c in range(3):
            nc.scalar.activation(xact[d][:, c * CH:(c + 1) * CH],
                                 pcs[c][:, 0:CH], AF.Silu,
                                 bias=cvb_t[:, d:d + 1])
        nc.vector.tensor_mul(xact[d][:, 0:130], xact[d][:, 0:130],
                             medge_b[:])

    inner = sb.tile([128, T_EXT], BF16, tag="inner", bufs=1, name="inner")
    inner_c = sb.tile([32, T_EXT], BF16, tag="inner_c", bufs=1, name="inner_c")
    pis = [psMM.tile([128, 512], F32, tag="pMM", name=f"pi{c}")
           for c in range(3)]
    for k in range(KT):
        for c in range(3):
            nc.tensor.matmul(pis[c][:, 0:CH], wig_t[:, k, :],
                             xact[k][:, c * CH:(c + 1) * CH],
                             start=(k == 0), stop=(k == KT - 1),
                             skip_group_check=True)
    for c in range(3):
        nc.vector.tensor_copy(inner[:, c * CH:(c + 1) * CH], pis[c][:, 0:CH])
        nc.scalar.copy(inner_c[:, c * CH:(c + 1) * CH],
                       pis[c][96:128, 0:CH])

    if DEBUG_DUMP:
        nc.sync.dma_start(t["dbg_xb"][:], xb[0][:].cast_dma(F32))
        nc.sync.dma_start(t["dbg_xact"][:], xact[0][:].cast_dma(F32))

    # ---- phase C: z half of in_proj (owned tokens only) + SiLU ----------
    for j in range(KT):
        pz = [psMM.tile([128, 512], F32, tag="pMM", name=f"pz{h}")
              for h in range(2)]
        for k in range(KT):
            for h in range(2):
                nc.tensor.matmul(pz[h][:], wz_t[:, k, j * 128:(j + 1) * 128],
                                 xn[k][:, OFF + h * 512:OFF + (h + 1) * 512],
                                 start=(k == 0), stop=(k == KT - 1),
                                 skip_group_check=True)
        for h in range(2):
            nc.scalar.activation(sz[j][:, h * 512:(h + 1) * 512], pz[h][:],
                                 AF.Silu)
        if DEBUG_DUMP and j == 0:
            nc.sync.dma_start(t["dbg_sz"][:], sz[j][:].cast_dma(F32))

    # ---- phase D: SSM per d-tile ----------------------------------------
    # Engine split per d: scalar = e/delta/dA (+ Cm eviction, Copy is in
    # every table set); DVE = dBx/scan/xds/g1 (the scan input path stays on
    # DVE to avoid cross-engine ping-pong); gpsimd = dx/Cz/final add.
    # dx/cm/cz/xds reuse the dead xb buffers (tag rotation).
    # out_proj for token blocks 0-1 accumulates per-k inside this loop (4
    # psMM banks held across the phase) so the PE has work as gated[k]
    # arrives and the phase-E tail shrinks.
    po01 = [psMM.tile([128, 512], F32, tag="pMM", name=f"po01_{i}")
            for i in range(4)]
    for d in range(KT):
        dsl = slice(d * 128, (d + 1) * 128)
        e_t = sb.tile([128, T_EXT], BF16, tag="e_t", bufs=2, name="e_t")
        for c in range(3):
            pd = psS.tile([128, 512], F32, tag="pS", name="pd")
            nc.tensor.matmul(pd[:, 0:CH], wab_t[0:64, dsl],
                             inner[0:64, c * CH:(c + 1) * CH],
                             start=True, stop=True)
            nc.scalar.activation(e_t[:, c * CH:(c + 1) * CH], pd[:, 0:CH],
                                 AF.Exp, bias=bdt_t[:, d:d + 1])
        # C-projection early; Cz = Cm*sz and xds = (xact*D)*sz are both
        # computed pre-scan so the post-scan chain is just two short ops.
        cm = sb.tile([128, OWN], BF16, tag="xb", bufs=KT, name="cm")
        for h in range(2):
            pc = psT.tile([128, 512], F32, tag="pT", name="pcm")
            nc.tensor.matmul(pc[:], wc_t[:, dsl],
                             inner_c[:, 128 + h * 512:128 + (h + 1) * 512],
                             start=True, stop=True)
            nc.scalar.copy(cm[:, h * 512:(h + 1) * 512], pc[:])
        cz = sb.tile([128, OWN], BF16, tag="xb", bufs=KT, name="cz")
        nc.gpsimd.tensor_mul(cz[:], cm[:], sz[d][:])
        xds = sb.tile([128, OWN], BF16, tag="xb", bufs=KT, name="xds")
        nc.vector.scalar_tensor_tensor(xds[:], xact[d][:, 128:T_EXT],
                                       D_t[:, d:d + 1], sz[d][:],
                                       OP.mult, OP.mult)
        delta = sb.tile([128, T_EXT], BF16, tag="delta", bufs=2, name="delta")
        nc.scalar.activation(delta[:], e_t[:], AF.Ln, bias=1.0)
        dA = sb.tile([128, T_EXT], BF16, tag="dA", bufs=2, name="dA")
        nc.scalar.activation(dA[:], delta[:], AF.Exp, scale=A_t[:, d:d + 1])

        dx = sb.tile([128, T_EXT], BF16, tag="xb", bufs=KT, name="dx")
        nc.vector.tensor_mul(dx[:], delta[:], xact[d][:])
        dBx = sb.tile([128, T_EXT], BF16, tag="dBx", bufs=2, name="dBx")
        for c in range(3):
            pb = psS.tile([128, 512], F32, tag="pS", name="pb")
            nc.tensor.matmul(pb[:, 0:CH], wab_t[64:96, dsl],
                             inner[64:96, c * CH:(c + 1) * CH],
                             start=True, stop=True)
            nc.vector.tensor_mul(dBx[:, c * CH:(c + 1) * CH], pb[:, 0:CH],
                                 dx[:, c * CH:(c + 1) * CH])

        hs = sb.tile([128, T_EXT], BF16, tag="hs", bufs=2, name="hs")
        nc.vector.tensor_tensor_scan(hs[:], dA[:], dBx[:], 0.0,
                                     OP.mult, OP.add)

        g1 = sb.tile([128, OWN], BF16, tag="ytmp", name="g1")
        nc.vector.tensor_mul(g1[:], cz[:], hs[:, 128:T_EXT])
        nc.gpsimd.tensor_tensor(gated[d][:], g1[:], xds[:], OP.add)

        # out_proj partial accumulation for token blocks 0-1 (k = d)
        for tb in range(2):
            for h in range(2):
                nc.tensor.matmul(po01[tb * 2 + h][:],
                                 gated[d][:, tb * 128:(tb + 1) * 128],
                                 wo_t[:, d, h * 512:(h + 1) * 512],
                                 start=(d == 0), stop=(d == KT - 1),
                                 skip_group_check=True)

    # ---- phase E: out_proj + residual -----------------------------------
    for tb in range(KT):
        po = po01[tb * 2:(tb + 1) * 2] if tb < 2 else [
            psMM.tile([128, 512], F32, tag="pMM", name=f"po{h}")
            for h in range(2)]
        if tb >= 2:
            for k in range(KT):
                for h in range(2):
                    nc.tensor.matmul(po[h][:],
                                     gated[k][:, tb * 128:(tb + 1) * 128],
                                     wo_t[:, k, h * 512:(h + 1) * 512],
                                     start=(k == 0), stop=(k == KT - 1),
                                     skip_group_check=True)
        for h in range(2):
            dq = nc.sync if h == 0 else nc.scalar
            xr = sb.tile([128, 512], F32, tag="xr", bufs=3, name="xr")
            dq.dma_start(
                xr[:], t["x_sl"][OFF + tb * 128:OFF + (tb + 1) * 128,
                                 h * 512:(h + 1) * 512])
            o_t = sb.tile([128, 512], F32, tag="o_t", bufs=3, name="o_t")
            nc.vector.tensor_tensor(o_t[:], po[h][:], xr[:], OP.add)
            dq.dma_start(
                t["out"][tb * 128:(tb + 1) * 128, h * 512:(h + 1) * 512],
                o_t[:])

    es.close()


# ---------------------------------------------------------------------------
# Host-side driver
# ---------------------------------------------------------------------------

_NC_CACHE = None


def _get_program():
    global _NC_CACHE
    if _NC_CACHE is None:
        _apply_patches()
        _NC_CACHE = build_program()
    return _NC_CACHE


def _prep_inputs(x, gamma, beta, W_in, conv_w, conv_b, W_xp, W_Bg, W_Cg,
                 W_dt, b_dt, W_Bp, W_Cp, A, D_skip, W_out):
    import ml_dtypes
    bf16 = ml_dtypes.bfloat16
    f = lambda v: np.ascontiguousarray(np.asarray(v, dtype=np.float32))
    bf = lambda v: np.ascontiguousarray(
        np.asarray(v, dtype=np.float32).astype(bf16))
    x = f(x)
    vec = lambda v: np.ascontiguousarray(f(v).reshape(KT, 128).T)  # [128, KT]

    w_ig = np.zeros((D, 128), np.float32)
    w_ig[:, 0:64] = f(W_xp)
    w_ig[:, 64:80] = f(W_Bg)
    w_ig[:, 96:112] = f(W_Cg)
    w_ab = np.zeros((128, D), np.float32)
    w_ab[0:64, :] = f(W_dt)
    w_ab[64:80, :] = f(W_Bp)
    w_c = np.zeros((32, D), np.float32)
    w_c[0:16, :] = f(W_Cp)

    convw = f(conv_w).reshape(KK, D)  # [3, 1024]
    convw_t = np.ascontiguousarray(
        convw.T.reshape(KT, 128, KK).transpose(1, 0, 2))  # [128, KT, 3]

    # pack [D, n] weights p-major: [128, KT*n] with partition data contiguous
    pmaj = lambda w: np.ascontiguousarray(
        w.reshape(KT, 128, -1).transpose(1, 0, 2).reshape(128, -1))
    shared = {
        "w_x": bf(pmaj(f(W_in[:, 0:D]))),
        "w_z": bf(pmaj(f(W_in[:, D:2 * D]))),
        "w_o": bf(pmaj(f(W_out))),
        "w_ig": bf(pmaj(w_ig)),
        "w_ab": bf(w_ab),
        "w_c": bf(w_c),
    }
    vpack = np.zeros((128, 81), np.float32)
    vpack[:, 0:8] = vec(gamma)
    vpack[:, 8:16] = vec(beta)
    vpack[:, 16:24] = vec(conv_b)
    vpack[:, 24:32] = vec(b_dt)
    vpack[:, 32:40] = vec(A)
    vpack[:, 40:48] = vec(D_skip)
    vpack[:, 48:72] = convw_t.reshape(128, 24)
    shared["vpack"] = vpack  # mask col filled per core below

    in_maps = []
    for core in range(8):
        b, q = divmod(core, 4)
        o = q * OWN
        lo = o - OFF
        xs = np.zeros((T_EXT, D), np.float32)
        mk = np.zeros((T_EXT,), np.float32)
        s_lo, s_hi = max(0, lo), min(L, lo + T_EXT)
        xs[s_lo - lo:s_hi - lo] = x[b, s_lo:s_hi]
        mk[s_lo - lo:s_hi - lo] = 1.0
        me = np.zeros((130,), np.float32)
        me[0] = 1.0 if 0 <= lo - 1 < L else 0.0
        me[1:130] = mk[0:129]
        vp = vpack.copy()
        vp[:, 72:81] = mk.reshape(9, 128).T
        in_maps.append({
            **shared,
            "x_sl": xs,
            "x_bf": xs.astype(bf16),
            "vpack": vp,
            "medge_row": me.reshape(1, 130),
            "mbl_row": mk[0:128].reshape(1, 128).copy(),
            "mbr_row": mk[OWN:T_EXT].reshape(1, 128).copy(),
        })
    return in_maps


def kernel(**inputs):
    from concourse.bass_utils import run_bass_kernel_spmd
    nc = _get_program()
    in_maps = _prep_inputs(**inputs)
    res = run_bass_kernel_spmd(nc, in_maps, core_ids=list(range(8)))
    out = np.empty((B, L, D), np.float32)
    for core in range(8):
        b, q = divmod(core, 4)
        out[b, q * OWN:(q + 1) * OWN, :] = res.results[core]["out"]
    return out


# revision 51
# speedup vs baseline: 23681.7687x; 1.0364x over previous
"""MambaVisionMixerBlock TRN2 Bass kernel (v2).

Sharding: 8 cores = 2 batches x 4 sequence-quarters. Each core owns 1024
tokens of one batch and computes the full block for them, using a 127-token
left halo so the selective scan's incoming state is reproduced to below
tolerance (decay exp(delta*A) per step; halo error decays ~0.88^127).

v2 layout/strategy (vs v1):
  - full 1152-token extent processed in one pass (scan init=0, no carries)
  - all weights resident in SBUF, bf16 (same PE rate, half the HBM/SBUF)
  - scalar-engine activations batched by table set (ln_exp -> silu -> ln_exp)
  - PSUM evictions distributed across DVE/gpsimd/scalar by phase load
  - LN interleaved with in_proj by 384-token chunks to keep the PE warm
  - residual applied on DVE; z-half computed for owned tokens only
"""

import json as _json

import numpy as np

import concourse.bass as bass
import concourse.mybir as mybir
import concourse.tile as tile
from concourse.masks import make_identity
from concourse.vector_clock import ScopedClock, VectorClock

F32 = mybir.dt.float32
F32R = mybir.dt.float32r
BF16 = mybir.dt.bfloat16
AF = mybir.ActivationFunctionType
OP = mybir.AluOpType

B, L, D = 2, 4096, 1024
DS, RK, KK = 16, 64, 3
EPS = 1e-5

T_EXT = 1152          # tokens per core incl halo
OWN = 1024            # owned tokens per core
OFF = 127             # owned ext cols = [127, 1151)
CH = 384              # ext-grid chunk width
KT = D // 128         # 8 d-tiles

DEBUG_DUMP = False

# ---------------------------------------------------------------------------
# Compiler workarounds: this container's walrus supports ONE sync-wait per
# instruction; Tile attaches several. Hoist extras onto single-wait NoOps.
# ---------------------------------------------------------------------------

_orig_to_json_bytes = bass.Bass.to_json_bytes


def _split_waits_json(raw: bytes) -> bytes:
    d = _json.loads(raw)
    changed = False
    for fn in d.get("functions", []):
        for bb in fn.get("blocks", []):
            out = []
            for inst in bb.get("instructions", []):
                si = inst.get("sync_info")
                waits = (si or {}).get("on_wait") or []
                if len(waits) > 1:
                    for i, w in enumerate(waits[:-1]):
                        out.append({
                            "debug": inst.get("debug", 0),
                            "engine": inst["engine"],
                            "ins": [],
                            "name": f"{inst['name']}-w{i}",
                            "opcode": "NoOp",
                            "outs": [],
                            "sync_info": {"on_update": [], "on_wait": [w]},
                        })
                    si["on_wait"] = [waits[-1]]
                    changed = True
                out.append(inst)
            bb["instructions"] = out
    if not changed:
        return raw
    return _json.dumps(d).encode()


def _patched_to_json_bytes(self, *a, **k):
    return _split_waits_json(_orig_to_json_bytes(self, *a, **k))


def _patched_drain_and_barrier(self, tick_clock, wait_clock):
    nc = self.nc
    gc = tick_clock.global_clock
    n_proc = len(gc)
    for proc in range(n_proc):
        tk = gc[proc]
        if tk > 0:
            vc = VectorClock([tk if i == proc else 0 for i in range(n_proc)])
            n = nc.sync.nop(nofuse=True)
            wait_clock.add_sem_waits(n.ins, ScopedClock({None: vc}))
    nc.sync.drain()
    nc.all_engine_barrier()
    assert self.sems is not None
    popped = nc._tile_sem_poison_stack.pop()
    assert popped is self._sem_poison
    nc.clear_and_free_semaphores(list(self.sems.allocated().values()))
    nc.all_engine_barrier()


def _apply_patches():
    bass.Bass.to_json_bytes = _patched_to_json_bytes
    tile.TileContext._drain_and_barrier = _patched_drain_and_barrier


# ---------------------------------------------------------------------------
# Program builder
# ---------------------------------------------------------------------------

def build_program():
    nc = bass.Bass("TRN2", target_bir_lowering=False, debug=False, num_devices=1)

    aps = {}

    def di(name, shape, dtype):
        aps[name] = nc.dram_tensor(name, shape, dtype, kind="ExternalInput").ap()

    di("x_sl", [T_EXT, D], F32)      # f32 x window (residual reads only)
    di("x_bf", [T_EXT, D], BF16)     # bf16 x window (LN reads; 2.3 MB not 4.7)
    di("w_x", [128, KT * D], BF16)   # in_proj xb half, p-major packed
    di("w_z", [128, KT * D], BF16)   # in_proj z half, p-major packed
    di("w_o", [128, KT * D], BF16)   # out_proj, p-major packed
    di("w_ig", [128, KT * 128], BF16)  # [W_xp | W_Bg pad | W_Cg pad], p-major
    di("w_ab", [128, D], BF16)       # rows [W_dt(64); W_Bp(16) pad to 96; 0]
    di("w_c", [32, D], BF16)         # rows [W_Cp(16); 0]
    # all small per-channel vectors packed into one DMA:
    # cols [0:8]=gamma [8:16]=beta [16:24]=convb [24:32]=bdt [32:40]=A
    # [40:48]=D [48:72]=convw(d*3+kk) [72:81]=mask_col
    di("vpack", [128, 81], F32)
    di("medge_row", [1, 130], F32)   # col c = mask of ext col c-1 (conv grid)
    di("mbl_row", [1, 128], F32)     # mask of ext cols [0,128)
    di("mbr_row", [1, 128], F32)     # mask of ext cols [1024,1152)

    aps["out"] = nc.dram_tensor("out", [OWN, D], F32, kind="ExternalOutput").ap()
    if DEBUG_DUMP:
        for nm, w in [("dbg_xn", T_EXT), ("dbg_xb", T_EXT + 2),
                      ("dbg_xact", T_EXT), ("dbg_delta", T_EXT),
                      ("dbg_dA", T_EXT), ("dbg_dBx", T_EXT),
                      ("dbg_hs", T_EXT), ("dbg_gated", OWN),
                      ("dbg_sz", OWN)]:
            aps[nm] = nc.dram_tensor(nm, [128, w], F32,
                                     kind="ExternalOutput").ap()

    with tile.TileContext(nc) as tc:
        _build_body(nc, tc, aps)
    return nc


def _build_body(nc, tc, t):
    from contextlib import ExitStack
    es = ExitStack()
    const = es.enter_context(tc.tile_pool(name="const", bufs=1))
    sb = es.enter_context(tc.tile_pool(name="sb", bufs=2))
    psT = es.enter_context(tc.tile_pool(name="psT", bufs=2, space="PSUM"))
    psMM = es.enter_context(tc.tile_pool(name="psMM", bufs=4, space="PSUM"))
    psS = es.enter_context(tc.tile_pool(name="psS", bufs=2, space="PSUM"))

    # ---- constants -------------------------------------------------------
    ident_f = const.tile([128, 128], F32, tag="ident_f")
    make_identity(nc, ident_f[:])
    ident_b = const.tile([128, 128], BF16, tag="ident_b")
    nc.vector.tensor_copy(ident_b[:], ident_f[:])

    def ldconst(name, shape, dtype, tag):
        tl = const.tile(shape, dtype, tag=tag, name=tag)
        nc.sync.dma_start(tl[:], t[name][:])
        return tl

    vpack_t = const.tile([128, 81], F32, tag="vpack", name="vpack")
    nc.scalar.dma_start(vpack_t[:], t["vpack"][:])
    gam_t = vpack_t[:, 0:8]
    bet_t = vpack_t[:, 8:16]
    cvb_t = vpack_t[:, 16:24]
    bdt_t = vpack_t[:, 24:32]
    A_t = vpack_t[:, 32:40]
    D_t = vpack_t[:, 40:48]
    convw_t = vpack_t[:, 48:72].rearrange("p (kt kk) -> p kt kk", kt=KT)
    mcol_t = vpack_t[:, 72:81]
    medge_row = ldconst("medge_row", [1, 130], F32, "merow")
    mbl_row = ldconst("mbl_row", [1, 128], F32, "mblrow")
    mbr_row = ldconst("mbr_row", [1, 128], F32, "mbrrow")
    ones1 = const.tile([1, 128], F32, tag="ones1")
    nc.gpsimd.memset(ones1[:], 1.0)
    eps_t = const.tile([128, 1], F32, tag="eps")
    nc.gpsimd.memset(eps_t[:], EPS)

    # resident weights (bf16, loaded once). Issued on the scalar (Activation)
    # HWDGE queue, in order of first use, so x/LN traffic on the sync queue
    # isn't stuck behind 6.6 MB of weights.
    wx_t = const.tile([128, KT, D], BF16, tag="wx")
    wig_t = const.tile([128, KT, 128], BF16, tag="wig")
    wz_t = const.tile([128, KT, D], BF16, tag="wz")
    wab_t = const.tile([128, D], BF16, tag="wab")
    nc.scalar.dma_start(wab_t[:], t["w_ab"][:])
    wc_t = const.tile([32, D], BF16, tag="wc")
    nc.scalar.dma_start(wc_t[:], t["w_c"][:])
    wo_t = const.tile([128, KT, D], BF16, tag="wo")

    # broadcast masks ([128, n] copies of row masks) via K=1 matmuls
    def bcast_mask(row_t, n, tag):
        ps = psMM.tile([128, 512], F32, tag="pMM", name=f"ps_{tag}")
        nc.tensor.matmul(ps[:, 0:n], ones1[:], row_t[:],
                         start=True, stop=True)
        mb = const.tile([128, n], F32, tag=tag)
        nc.vector.tensor_copy(mb[:], ps[:, 0:n])
        return mb

    # ---- persistent activations -----------------------------------------
    xn = [sb.tile([128, T_EXT], BF16, tag="xn", bufs=KT, name=f"xn{d}")
          for d in range(KT)]
    xb = [sb.tile([128, T_EXT + 2], BF16, tag="xb", bufs=KT, name=f"xb{d}")
          for d in range(KT)]
    xact = [sb.tile([128, T_EXT], BF16, tag="xact", bufs=KT, name=f"xact{d}")
            for d in range(KT)]
    sz = [sb.tile([128, OWN], BF16, tag="sz", bufs=KT, name=f"sz{d}")
          for d in range(KT)]
    gated = [sb.tile([128, OWN], BF16, tag="gated", bufs=KT, name=f"gated{d}")
             for d in range(KT)]
    for d in range(KT):
        nc.vector.memset(xb[d][:, 0:2], 0.0)

    # ---- phase A: LN + transpose + in_proj xb, by 384-token chunk -------
    def ln_chunk(g):
        if g == 1:
            nc.scalar.dma_start(
                wz_t[:], t["w_z"].rearrange("p (kt j) -> p kt j", kt=KT))
        if g == 2:
            nc.scalar.dma_start(
                wo_t[:], t["w_o"].rearrange("p (kt j) -> p kt j", kt=KT))
        xhat = []
        for tt in range(3):
            ts0 = g * CH + tt * 128
            x_tm = sb.tile([128, D], BF16, tag="x_tm", name="x_tm")
            (nc.scalar if g == 0 else nc.sync).dma_start(
                x_tm[:], t["x_bf"][ts0:ts0 + 128, :])
            if g == 0 and tt == 2:
                nc.scalar.dma_start(
                    wx_t[:], t["w_x"].rearrange("p (kt j) -> p kt j", kt=KT))
                nc.scalar.dma_start(
                    wig_t[:], t["w_ig"].rearrange("p (kt j) -> p kt j", kt=KT))
            scratch = sb.tile([128, D], BF16, tag="scr", bufs=1, name="scr")
            sx = sb.tile([128, 1], F32, tag="sx", name="sx")
            nc.vector.tensor_reduce(sx[:], x_tm[:], mybir.AxisListType.XYZW,
                                    OP.add)
            sq = sb.tile([128, 1], F32, tag="sq", name="sq")
            nc.scalar.activation(scratch[:], x_tm[:], AF.Square,
                                 accum_out=sq[:])
            negmu = sb.tile([128, 1], F32, tag="negmu", name="negmu")
            nc.vector.tensor_scalar(negmu[:], sx[:], -1.0 / D, None, OP.mult)
            mu = sb.tile([128, 1], F32, tag="mu", name="mu")
            nc.vector.tensor_scalar(mu[:], sx[:], 1.0 / D, None, OP.mult)
            msq = sb.tile([128, 1], F32, tag="msq", name="msq")
            nc.vector.tensor_scalar(msq[:], sq[:], 1.0 / D, None, OP.mult)
            var = sb.tile([128, 1], F32, tag="var", name="var")
            nc.vector.scalar_tensor_tensor(var[:], mu[:], negmu[:], msq[:],
                                           OP.mult, OP.add)
            lnv = sb.tile([128, 1], F32, tag="lnv", name="lnv")
            nc.scalar.activation(lnv[:], var[:], AF.Ln, bias=eps_t[:])
            sinv = sb.tile([128, 1], F32, tag="sinv", name="sinv")
            nc.scalar.activation(sinv[:], lnv[:], AF.Exp, scale=-0.5)
            mi = g * 3 + tt
            sc_eff = sb.tile([128, 1], F32, tag="sc_eff", name="sc_eff")
            nc.vector.tensor_mul(sc_eff[:], sinv[:], mcol_t[:, mi:mi + 1])
            bi_eff = sb.tile([128, 1], F32, tag="bi_eff", name="bi_eff")
            nc.vector.tensor_mul(bi_eff[:], negmu[:], sc_eff[:])
            xh = sb.tile([128, D], BF16, tag="xhat", bufs=3, name="xhat")
            nc.vector.tensor_scalar(xh[:], x_tm[:], sc_eff[:], bi_eff[:],
                                    OP.mult, OP.add)
            xhat.append(xh)
        return xhat

    def transpose_one(g, xhat, d):
        # channel-major transpose + gamma/beta for one d-tile of chunk g
        ps_x = psT.tile([128, CH], BF16, tag="pT", name="ps_x")
        for tt in range(3):
            nc.tensor.matmul(ps_x[:, tt * 128:(tt + 1) * 128],
                             xhat[tt][:, d * 128:(d + 1) * 128],
                             ident_b[:], is_transpose=True,
                             start=True, stop=True,
                             skip_group_check=True)
        nc.vector.tensor_scalar(xn[d][:, g * CH:(g + 1) * CH], ps_x[:],
                                gam_t[:, d:d + 1], bet_t[:, d:d + 1],
                                OP.mult, OP.add)

    def inproj_one(g, j):
        ps = psMM.tile([128, 512], F32, tag="pMM", name="ps_xb")
        for k in range(KT):
            nc.tensor.matmul(ps[:, 0:CH], wx_t[:, k, j * 128:(j + 1) * 128],
                             xn[k][:, g * CH:(g + 1) * CH],
                             start=(k == 0), stop=(k == KT - 1),
                             skip_group_check=True)
        nc.vector.tensor_copy(xb[j][:, 2 + g * CH:2 + (g + 1) * CH],
                              ps[:, 0:CH])

    # chunk 0: LN + transposes up front; chunks 1-2: the next chunk's
    # transposes are interleaved into the current chunk's in_proj j-loop in
    # <=2-tile bursts so transpose-mode (which HAM ignores) never forms a
    # >3.4us "idle" window that re-throttles the PE clock.
    xhat_g = ln_chunk(0)
    for d in range(KT):
        transpose_one(0, xhat_g, d)
    for g in range(3):
        xhat_next = ln_chunk(g + 1) if g < 2 else None
        for j in range(KT):
            inproj_one(g, j)
            if xhat_next is not None and j % 2 == 1:
                d0 = (j // 2) * 2
                transpose_one(g + 1, xhat_next, d0)
                transpose_one(g + 1, xhat_next, d0 + 1)

    # conv diagonal weights (scalar-engine Copy with per-partition scale;
    # Copy is in every ACT table set so this causes no table reload)
    diag = []
    for kk in range(KK):
        row = []
        for d in range(KT):
            dg = const.tile([128, 128], BF16, tag=f"diag{kk}_{d}",
                            name=f"diag{kk}_{d}")
            nc.scalar.activation(dg[:], ident_f[:], AF.Copy,
                                 scale=convw_t[:, d, kk:kk + 1])
            row.append(dg)
        diag.append(row)

    medge_b = bcast_mask(medge_row, 130, "medgeb")
    mbl_b = bcast_mask(mbl_row, 128, "mblb")
    mbr_b = bcast_mask(mbr_row, 128, "mbrb")

    # pad-edge zeroing of xn (general-beta safety; in-place DVE muls)
    for d in range(KT):
        nc.vector.tensor_mul(xn[d][:, 0:128], xn[d][:, 0:128], mbl_b[:])
        nc.vector.tensor_mul(xn[d][:, OWN:T_EXT], xn[d][:, OWN:T_EXT],
                             mbr_b[:])
        if DEBUG_DUMP and d == 0:
            nc.sync.dma_start(t["dbg_xn"][:], xn[d][:].cast_dma(F32))

    # ---- phase B: depthwise conv + SiLU, then inner projections ---------
    for d in range(KT):
        pcs = [psMM.tile([128, 512], F32, tag="pMM", name=f"pc{c}")
               for c in range(3)]
        for kk in range(KK):
            for c in range(3):
                nc.tensor.matmul(pcs[c][:, 0:CH], diag[kk][d][:],
                                 xb[d][:, c * CH + kk:c * CH + kk + CH],
                                 start=(kk == 0), stop=(kk == KK - 1),
                                 skip_group_check=True)
        for c in range(3):
            nc.scalar.activation(xact[d][:, c * CH:(c + 1) * CH],
                                 pcs[c][:, 0:CH], AF.Silu,
                                 bias=cvb_t[:, d:d + 1])
        nc.vector.tensor_mul(xact[d][:, 0:130], xact[d][:, 0:130],
                             medge_b[:])

    inner = sb.tile([128, T_EXT], BF16, tag="inner", bufs=1, name="inner")
    inner_c = sb.tile([32, T_EXT], BF16, tag="inner_c", bufs=1, name="inner_c")
    pis = [psMM.tile([128, 512], F32, tag="pMM", name=f"pi{c}")
           for c in range(3)]
    for k in range(KT):
        for c in range(3):
            nc.tensor.matmul(pis[c][:, 0:CH], wig_t[:, k, :],
                             xact[k][:, c * CH:(c + 1) * CH],
                             start=(k == 0), stop=(k == KT - 1),
                             skip_group_check=True)
    for c in range(3):
        nc.vector.tensor_copy(inner[:, c * CH:(c + 1) * CH], pis[c][:, 0:CH])
        nc.scalar.copy(inner_c[:, c * CH:(c + 1) * CH],
                       pis[c][96:128, 0:CH])

    if DEBUG_DUMP:
        nc.sync.dma_start(t["dbg_xb"][:], xb[0][:].cast_dma(F32))
        nc.sync.dma_start(t["dbg_xact"][:], xact[0][:].cast_dma(F32))

    # ---- phase C: z half of in_proj (owned tokens only) + SiLU ----------
    for j in range(KT):
        pz = [psMM.tile([128, 512], F32, tag="pMM", name=f"pz{h}")
              for h in range(2)]
        for k in range(KT):
            for h in range(2):
                nc.tensor.matmul(pz[h][:], wz_t[:, k, j * 128:(j + 1) * 128],
                                 xn[k][:, OFF + h * 512:OFF + (h + 1) * 512],
                                 start=(k == 0), stop=(k == KT - 1),
                                 skip_group_check=True)
        for h in range(2):
            nc.scalar.activation(sz[j][:, h * 512:(h + 1) * 512], pz[h][:],
                                 AF.Silu)
        if DEBUG_DUMP and j == 0:
            nc.sync.dma_start(t["dbg_sz"][:], sz[j][:].cast_dma(F32))

    # ---- phase D: SSM per d-tile ----------------------------------------
    # Engine split per d: scalar = e/delta/dA (+ Cm eviction, Copy is in
    # every table set); DVE = dBx/scan/xds/g1 (the scan input path stays on
    # DVE to avoid cross-engine ping-pong); gpsimd = dx/Cz/final add.
    # dx/cm/cz/xds reuse the dead xb buffers (tag rotation).
    # out_proj for token blocks 0-1 accumulates per-k inside this loop (4
    # psMM banks held across the phase) so the PE has work as gated[k]
    # arrives and the phase-E tail shrinks.
    po01 = [psMM.tile([128, 512], F32, tag="pMM", name=f"po01_{i}")
            for i in range(4)]
    for d in range(KT):
        dsl = slice(d * 128, (d + 1) * 128)
        e_t = sb.tile([128, T_EXT], BF16, tag="e_t", bufs=2, name="e_t")
        for c in range(3):
            pd = psS.tile([128, 512], F32, tag="pS", name="pd")
            nc.tensor.matmul(pd[:, 0:CH], wab_t[0:64, dsl],
                             inner[0:64, c * CH:(c + 1) * CH],
                             start=True, stop=True)
            nc.scalar.activation(e_t[:, c * CH:(c + 1) * CH], pd[:, 0:CH],
                                 AF.Exp, bias=bdt_t[:, d:d + 1])
        # C-projection early; Cz = Cm*sz and xds = (xact*D)*sz are both
        # computed pre-scan so the post-scan chain is just two short ops.
        cm = sb.tile([128, OWN], BF16, tag="xb", bufs=KT, name="cm")
        for h in range(2):
            pc = psT.tile([128, 512], F32, tag="pT", name="pcm")
            nc.tensor.matmul(pc[:], wc_t[:, dsl],
                             inner_c[:, 128 + h * 512:128 + (h + 1) * 512],
                             start=True, stop=True)
            nc.scalar.copy(cm[:, h * 512:(h + 1) * 512], pc[:])
        cz = sb.tile([128, OWN], BF16, tag="xb", bufs=KT, name="cz")
        nc.gpsimd.tensor_mul(cz[:], cm[:], sz[d][:])
        xds = sb.tile([128, OWN], BF16, tag="xb", bufs=KT, name="xds")
        nc.vector.scalar_tensor_tensor(xds[:], xact[d][:, 128:T_EXT],
                                       D_t[:, d:d + 1], sz[d][:],
                                       OP.mult, OP.mult)
        delta = sb.tile([128, T_EXT], BF16, tag="delta", bufs=2, name="delta")
        nc.scalar.activation(delta[:], e_t[:], AF.Ln, bias=1.0)
        dA = sb.tile([128, T_EXT], BF16, tag="dA", bufs=2, name="dA")
        nc.scalar.activation(dA[:], delta[:], AF.Exp, scale=A_t[:, d:d + 1])

        dx = sb.tile([128, T_EXT], BF16, tag="xb", bufs=KT, name="dx")
        nc.vector.tensor_mul(dx[:], delta[:], xact[d][:])
        dBx = sb.tile([128, T_EXT], BF16, tag="dBx", bufs=2, name="dBx")
        for c in range(3):
            pb = psS.tile([128, 512], F32, tag="pS", name="pb")
            nc.tensor.matmul(pb[:, 0:CH], wab_t[64:96, dsl],
                             inner[64:96, c * CH:(c + 1) * CH],
                             start=True, stop=True)
            nc.vector.tensor_mul(dBx[:, c * CH:(c + 1) * CH], pb[:, 0:CH],
                                 dx[:, c * CH:(c + 1) * CH])

        hs = sb.tile([128, T_EXT], BF16, tag="hs", bufs=2, name="hs")
        nc.vector.tensor_tensor_scan(hs[:], dA[:], dBx[:], 0.0,
                                     OP.mult, OP.add)

        g1 = sb.tile([128, OWN], BF16, tag="ytmp", name="g1")
        nc.vector.tensor_mul(g1[:], cz[:], hs[:, 128:T_EXT])
        nc.gpsimd.tensor_tensor(gated[d][:], g1[:], xds[:], OP.add)

        # out_proj partial accumulation for token blocks 0-1 (k = d)
        for tb in range(2):
            for h in range(2):
                nc.tensor.matmul(po01[tb * 2 + h][:],
                                 gated[d][:, tb * 128:(tb + 1) * 128],
                                 wo_t[:, d, h * 512:(h + 1) * 512],
                                 start=(d == 0), stop=(d == KT - 1),
                                 skip_group_check=True)

    # ---- phase E: out_proj + residual -----------------------------------
    for tb in range(KT):
        po = po01[tb * 2:(tb + 1) * 2] if tb < 2 else [
            psMM.tile([128, 512], F32, tag="pMM", name=f"po{h}")
            for h in range(2)]
        if tb >= 2:
            for k in range(KT):
                for h in range(2):
                    nc.tensor.matmul(po[h][:],
                                     gated[k][:, tb * 128:(tb + 1) * 128],
                                     wo_t[:, k, h * 512:(h + 1) * 512],
                                     start=(k == 0), stop=(k == KT - 1),
                                     skip_group_check=True)
        for h in range(2):
            # residual read from the bf16 x copy on the sync queue (deep
            # prefetch); out writes go on the scalar queue so the write's
            # wait-for-o_t never blocks the next residual read.
            xr = sb.tile([128, 512], BF16, tag="xr", bufs=6, name="xr")
            nc.sync.dma_start(
                xr[:], t["x_bf"][OFF + tb * 128:OFF + (tb + 1) * 128,
                                 h * 512:(h + 1) * 512])
            o_t = sb.tile([128, 512], F32, tag="o_t", bufs=3, name="o_t")
            nc.vector.tensor_tensor(o_t[:], po[h][:], xr[:], OP.add)
            nc.scalar.dma_start(
                t["out"][tb * 128:(tb + 1) * 128, h * 512:(h + 1) * 512],
                o_t[:])

    es.close()


# ---------------------------------------------------------------------------
# Host-side driver
# ---------------------------------------------------------------------------

_NC_CACHE = None


def _get_program():
    global _NC_CACHE
    if _NC_CACHE is None:
        _apply_patches()
        _NC_CACHE = build_program()
    return _NC_CACHE


def _prep_inputs(x, gamma, beta, W_in, conv_w, conv_b, W_xp, W_Bg, W_Cg,
                 W_dt, b_dt, W_Bp, W_Cp, A, D_skip, W_out):
    import ml_dtypes
    bf16 = ml_dtypes.bfloat16
    f = lambda v: np.ascontiguousarray(np.asarray(v, dtype=np.float32))
    bf = lambda v: np.ascontiguousarray(
        np.asarray(v, dtype=np.float32).astype(bf16))
    x = f(x)
    vec = lambda v: np.ascontiguousarray(f(v).reshape(KT, 128).T)  # [128, KT]

    w_ig = np.zeros((D, 128), np.float32)
    w_ig[:, 0:64] = f(W_xp)
    w_ig[:, 64:80] = f(W_Bg)
    w_ig[:, 96:112] = f(W_Cg)
    w_ab = np.zeros((128, D), np.float32)
    w_ab[0:64, :] = f(W_dt)
    w_ab[64:80, :] = f(W_Bp)
    w_c = np.zeros((32, D), np.float32)
    w_c[0:16, :] = f(W_Cp)

    convw = f(conv_w).reshape(KK, D)  # [3, 1024]
    convw_t = np.ascontiguousarray(
        convw.T.reshape(KT, 128, KK).transpose(1, 0, 2))  # [128, KT, 3]

    # pack [D, n] weights p-major: [128, KT*n] with partition data contiguous
    pmaj = lambda w: np.ascontiguousarray(
        w.reshape(KT, 128, -1).transpose(1, 0, 2).reshape(128, -1))
    shared = {
        "w_x": bf(pmaj(f(W_in[:, 0:D]))),
        "w_z": bf(pmaj(f(W_in[:, D:2 * D]))),
        "w_o": bf(pmaj(f(W_out))),
        "w_ig": bf(pmaj(w_ig)),
        "w_ab": bf(w_ab),
        "w_c": bf(w_c),
    }
    vpack = np.zeros((128, 81), np.float32)
    vpack[:, 0:8] = vec(gamma)
    vpack[:, 8:16] = vec(beta)
    vpack[:, 16:24] = vec(conv_b)
    vpack[:, 24:32] = vec(b_dt)
    vpack[:, 32:40] = vec(A)
    vpack[:, 40:48] = vec(D_skip)
    vpack[:, 48:72] = convw_t.reshape(128, 24)
    shared["vpack"] = vpack  # mask col filled per core below

    in_maps = []
    for core in range(8):
        b, q = divmod(core, 4)
        o = q * OWN
        lo = o - OFF
        xs = np.zeros((T_EXT, D), np.float32)
        mk = np.zeros((T_EXT,), np.float32)
        s_lo, s_hi = max(0, lo), min(L, lo + T_EXT)
        xs[s_lo - lo:s_hi - lo] = x[b, s_lo:s_hi]
        mk[s_lo - lo:s_hi - lo] = 1.0
        me = np.zeros((130,), np.float32)
        me[0] = 1.0 if 0 <= lo - 1 < L else 0.0
        me[1:130] = mk[0:129]
        vp = vpack.copy()
        vp[:, 72:81] = mk.reshape(9, 128).T
        in_maps.append({
            **shared,
            "x_sl": xs,
            "x_bf": xs.astype(bf16),
            "vpack": vp,
            "medge_row": me.reshape(1, 130),
            "mbl_row": mk[0:128].reshape(1, 128).copy(),
            "mbr_row": mk[OWN:T_EXT].reshape(1, 128).copy(),
        })
    return in_maps


def kernel(**inputs):
    from concourse.bass_utils import run_bass_kernel_spmd
    nc = _get_program()
    in_maps = _prep_inputs(**inputs)
    res = run_bass_kernel_spmd(nc, in_maps, core_ids=list(range(8)))
    out = np.empty((B, L, D), np.float32)
    for core in range(8):
        b, q = divmod(core, 4)
        out[b, q * OWN:(q + 1) * OWN, :] = res.results[core]["out"]
    return out
